# revision 15
# baseline (speedup 1.0000x reference)
"""BPBookMemory retrieval-knn kernel for 8 Trainium2 NeuronCores.

Pipeline per core (data-parallel over batch: core b handles x[b]):
  1. Normalize memory rows, transpose to memT [128, 4, 8192] (SBUF-resident).
  2. Per 128-row tile of x: normalize, PE-transpose, sim = x_n @ mem_n^T via
     float32r (TF32) matmuls into PSUM, copy rows to SBUF.
  3. DVE max/max_index -> top-8 values + indices per row.
  4. ACT softmax over 8 scores (scale folded in).
  5. gpsimd indirect-DMA gather of memory rows, ACT per-k scaling,
     gpsimd tree-add, final residual add, DMA out.
"""

import sys

import numpy as np

for _p in ("/opt/trn_rl_repo",):
    if _p not in sys.path:
        sys.path.insert(0, _p)

P = 128
D = 512
S = 8192
K = 8
ROWS = 4096  # rows per core (one batch element)
DC = D // P  # 4 contraction chunks
NCH = S // 512  # 16 sim column chunks
B = 8

_cache = {}
last_results = None  # BassKernelResults of most recent kernel() call (for test harness)


def build_program(scale: float, n_row_tiles: int = ROWS // P):
    import concourse.bass as bass
    import concourse.mybir as mybir
    import concourse.tile as tile
    from concourse import bacc
    from concourse.masks import make_identity

    f32 = mybir.dt.float32
    f32r = mybir.dt.float32r
    u32 = mybir.dt.uint32
    AF = mybir.ActivationFunctionType
    OP = mybir.AluOpType

    nc = bacc.Bacc()
    x_in = nc.dram_tensor("x", [n_row_tiles * P, D], f32, kind="ExternalInput")
    mem_in = nc.dram_tensor("memory", [S, D], f32, kind="ExternalInput")
    out_t = nc.dram_tensor("out", [n_row_tiles * P, D], f32, kind="ExternalOutput")

    with tile.TileContext(nc) as tc:
        with (
            tc.tile_pool(name="const", bufs=1) as const_pool,
            tc.tile_pool(name="mwork", bufs=2) as mwork,
            tc.tile_pool(name="xwork", bufs=2) as xwork,
            tc.tile_pool(name="sim", bufs=1) as sim_pool,
            tc.tile_pool(name="retr", bufs=1) as retr_pool,
            tc.tile_pool(name="small", bufs=4) as small,
            tc.tile_pool(name="psum_t", bufs=2, space="PSUM") as psum_t,
            tc.tile_pool(name="psum_sim", bufs=4, space="PSUM") as psum_sim,
        ):
            ident = const_pool.tile([P, P], f32)
            make_identity(nc, ident[:])
            # PE instructions carry at most ONE sync wait. Absorb the Pool
            # (make_identity) dependency into PE's observed clock once via a
            # dummy transpose, and keep every other PE input producer / PSUM
            # consumer on ACT so each matmul needs only the ACT wait.
            scratch = const_pool.tile([P, P], f32)
            pt0 = psum_t.tile([P, P], f32, tag="pt")
            nc.tensor.transpose(pt0[:], ident[:], ident[:])
            nc.scalar.copy(scratch[:], pt0[:])

            memT = const_pool.tile([P, DC, S], f32r)

            # ---- prologue: normalize + transpose memory table ----
            for s in range(S // P):
                mt = mwork.tile([P, D], f32, tag="mt")
                nc.sync.dma_start(mt[:], mem_in[s * P : (s + 1) * P, :])
                mn = mwork.tile([P, D], f32, tag="mn")
                ss = mwork.tile([P, 1], f32, tag="mss")
                nc.scalar.activation(mn[:], mt[:], AF.Square, accum_out=ss[:])
                nrm = mwork.tile([P, 1], f32, tag="mnrm")
                nc.scalar.sqrt(nrm[:], ss[:])
                inv = mwork.tile([P, 1], f32, tag="minv")
                nc.vector.reciprocal(inv[:], nrm[:])
                nc.scalar.activation(mn[:], mt[:], AF.Copy, scale=inv[:, 0:1])
                for d in range(DC):
                    pt = psum_t.tile([P, P], f32, tag="pt")
                    nc.tensor.transpose(pt[:], mn[:, d * P : (d + 1) * P], ident[:])
                    nc.scalar.copy(memT[:, d, s * P : (s + 1) * P], pt[:])

            # ---- main loop over row tiles ----
            for m in range(n_row_tiles):
                xt = xwork.tile([P, D], f32, tag="xt")
                nc.sync.dma_start(xt[:], x_in[m * P : (m + 1) * P, :])
                xn = xwork.tile([P, D], f32, tag="xn")
                xss = small.tile([P, 1], f32, tag="xss")
                nc.scalar.activation(xn[:], xt[:], AF.Square, accum_out=xss[:])
                xnrm = small.tile([P, 1], f32, tag="xnrm")
                nc.scalar.sqrt(xnrm[:], xss[:])
                xinv = small.tile([P, 1], f32, tag="xinv")
                nc.vector.reciprocal(xinv[:], xnrm[:])
                nc.scalar.activation(xn[:], xt[:], AF.Copy, scale=xinv[:, 0:1])
                xT = xwork.tile([P, DC, P], f32r, tag="xT")
                for d in range(DC):
                    pt = psum_t.tile([P, P], f32, tag="pt")
                    nc.tensor.transpose(pt[:], xn[:, d * P : (d + 1) * P], ident[:])
                    nc.scalar.copy(xT[:, d, :], pt[:])

                simrow = sim_pool.tile([P, S], f32, tag="simrow")
                for n in range(NCH):
                    ps = psum_sim.tile([P, 512], f32, tag="ps")
                    for d in range(DC):
                        nc.tensor.matmul(
                            ps[:],
                            lhsT=xT[:, d, :],
                            rhs=memT[:, d, n * 512 : (n + 1) * 512],
                            start=(d == 0),
                            stop=(d == DC - 1),
                        )
                    nc.scalar.copy(simrow[:, n * 512 : (n + 1) * 512], ps[:])

                vals = small.tile([P, K], f32, tag="vals")
                nc.vector.max(out=vals[:], in_=simrow[:])
                idx = small.tile([P, K], u32, tag="idx")
                nc.vector.max_index(out=idx[:], in_max=vals[:], in_values=simrow[:])

                negmax = small.tile([P, 1], f32, tag="negmax")
                nc.vector.tensor_scalar_mul(negmax[:], vals[:, 0:1], -1.0)
                e8 = small.tile([P, K], f32, tag="e8")
                zsum = small.tile([P, 1], f32, tag="zsum")
                nc.scalar.activation(
                    e8[:], vals[:], AF.Exp, bias=negmax[:, 0:1], accum_out=zsum[:]
                )
                rz = small.tile([P, 1], f32, tag="rz")
                nc.vector.reciprocal(rz[:], zsum[:])
                w8 = small.tile([P, K], f32, tag="w8")
                nc.vector.tensor_scalar(
                    w8[:], e8[:], rz[:, 0:1], float(scale), op0=OP.mult, op1=OP.mult
                )

                retr = retr_pool.tile([P, K, D], f32, tag="retr")
                for k in range(K):
                    nc.gpsimd.indirect_dma_start(
                        out=retr[:, k, :],
                        out_offset=None,
                        in_=mem_in[:],
                        in_offset=bass.IndirectOffsetOnAxis(ap=idx[:, k : k + 1], axis=0),
                    )
                for k in range(K):
                    nc.scalar.activation(
                        retr[:, k, :], retr[:, k, :], AF.Copy, scale=w8[:, k : k + 1]
                    )
                nc.gpsimd.tensor_add(retr[:, 0:4, :], retr[:, 0:4, :], retr[:, 4:8, :])
                nc.gpsimd.tensor_add(retr[:, 0:2, :], retr[:, 0:2, :], retr[:, 2:4, :])
                nc.gpsimd.tensor_add(retr[:, 0:1, :], retr[:, 0:1, :], retr[:, 1:2, :])
                nc.gpsimd.tensor_add(xt[:], xt[:], retr[:, 0, :])
                nc.sync.dma_start(out_t[m * P : (m + 1) * P, :], xt[:])

    return nc


def _get_program(scale: float, n_row_tiles: int = ROWS // P):
    key = (float(scale), n_row_tiles)
    if key not in _cache:
        nc = build_program(float(scale), n_row_tiles)
        if not nc.is_finalized():
            nc.finalize()  # runs the Bacc lowering pipeline (wait splitting etc.)
        _cache[key] = nc
    return _cache[key]


def kernel(x: np.ndarray, memory: np.ndarray, retrieval_scale) -> np.ndarray:
    from concourse.bass_utils import run_bass_kernel_spmd

    x = np.ascontiguousarray(x, dtype=np.float32)
    memory = np.ascontiguousarray(memory, dtype=np.float32)
    scale = float(np.asarray(retrieval_scale))

    nc = _get_program(scale)
    in_maps = [
        {"x": x[b].reshape(ROWS, D), "memory": memory} for b in range(x.shape[0])
    ]
    res = run_bass_kernel_spmd(nc, in_maps, core_ids=list(range(x.shape[0])))
    global last_results
    last_results = res
    out = np.stack([r["out"] for r in res.results]).reshape(x.shape)
    return out.astype(np.float32)


# revision 28
# speedup vs baseline: 1.1184x; 1.1184x over previous
"""BPBookMemory retrieval-knn kernel for 8 Trainium2 NeuronCores.

Per core (data-parallel over batch: core b handles x[b] [4096, 512]):
  1. Prologue: row inv-norms of x and memory (DVE tensor_tensor_reduce +
     one sqrt + one reciprocal); normalized memory transposed into fp16
     memT [128, 4, 8192] (SBUF-resident).
  2. Per 128-row tile of x: PE-transpose raw x (ranking is invariant to
     row scale), sim_raw = x @ mem_n^T via fp16 matmuls (fp16 mantissa ==
     tf32 mantissa, but FWL makes LDWEIGHTS ~free), ACT copies PSUM->SBUF.
  3. DVE max/max_index -> top-8 raw scores + indices per row.
  4. ACT softmax with 1/||x|| folded into the exp scale.
  5. One merged gpsimd indirect-DMA gather of 8 memory rows per row,
     ACT per-k scaling, gpsimd tree-add + residual add, DMA out.
"""

import sys

import numpy as np

for _p in ("/opt/trn_rl_repo",):
    if _p not in sys.path:
        sys.path.insert(0, _p)

P = 128
D = 512
S = 8192
K = 8
ROWS = 4096  # rows per core (one batch element)
DC = D // P  # 4 contraction chunks
NCH = S // 512  # 16 sim column chunks
B = 8
MERGED_GATHER = False
USE_F16 = True  # fp16 matmul operands (vs float32r)
F16_TRANSPOSE = True  # transpose memory tiles in fp16 (vs f32 + cast on copy)
SIM_BUFS = 2
RETR_BUFS = 2
NWORK_BUFS = 3

_cache = {}
last_results = None  # BassKernelResults of most recent kernel() call


def build_program(scale: float, n_row_tiles: int = ROWS // P):
    import concourse.bass as bass
    import concourse.mybir as mybir
    import concourse.tile as tile
    from concourse import bacc
    from concourse.masks import make_identity

    f32 = mybir.dt.float32
    f16 = mybir.dt.float16 if USE_F16 else mybir.dt.float32r
    f16_t = mybir.dt.float16 if (USE_F16 and F16_TRANSPOSE) else mybir.dt.float32
    u32 = mybir.dt.uint32
    AF = mybir.ActivationFunctionType
    OP = mybir.AluOpType
    MT = n_row_tiles

    nc = bacc.Bacc()
    x_in = nc.dram_tensor("x", [MT * P, D], f32, kind="ExternalInput")
    mem_in = nc.dram_tensor("memory", [S, D], f32, kind="ExternalInput")
    out_t = nc.dram_tensor("out", [MT * P, D], f32, kind="ExternalOutput")

    with tile.TileContext(nc) as tc:
        with (
            tc.tile_pool(name="const", bufs=1) as const_pool,
            tc.tile_pool(name="xwork", bufs=2) as xwork,
            tc.tile_pool(name="sim", bufs=SIM_BUFS) as sim_pool,
            tc.tile_pool(name="retr", bufs=RETR_BUFS) as retr_pool,
            tc.tile_pool(name="small", bufs=4) as small,
            tc.tile_pool(name="psum_t", bufs=2, space="PSUM") as psum_t,
            tc.tile_pool(name="psum_t16", bufs=2, space="PSUM") as psum_t16,
            tc.tile_pool(name="psum_sim", bufs=4, space="PSUM") as psum_sim,
        ):
            ident = const_pool.tile([P, P], f32)
            make_identity(nc, ident[:])
            ident16 = const_pool.tile([P, P], f16_t)
            nc.vector.tensor_copy(ident16[:], ident[:])
            # PE instructions carry at most ONE wait cheaply; absorb the Pool
            # (make_identity) dep into PE's observed clock via dummy transposes.
            scratch = const_pool.tile([P, P], f32)
            pt0 = psum_t.tile([P, P], f32, tag="pt")
            nc.tensor.transpose(pt0[:], ident[:], ident[:])
            nc.scalar.copy(scratch[:], pt0[:])
            scratch16 = const_pool.tile([P, P], f16_t)
            pt016 = psum_t16.tile([P, P], f16_t, tag="pt16")
            nc.tensor.transpose(pt016[:], ident16[:], ident16[:])
            nc.scalar.copy(scratch16[:], pt016[:])

            memT = const_pool.tile([P, DC, S], f16)
            invx = const_pool.tile([P, MT], f32)
            invm = const_pool.tile([P, S // P], f32)

            # ---- prologue A: x row inv-norms (extra cheap pass over x) ----
            with tc.tile_pool(name="nwork", bufs=NWORK_BUFS) as nwork:
                ssx = const_pool.tile([P, MT], f32)
                for m in range(MT):
                    xt0 = nwork.tile([P, D], f32, tag="nload")
                    nc.sync.dma_start(xt0[:], x_in[m * P : (m + 1) * P, :])
                    sqsc = nwork.tile([P, D], f32, tag="sqscr")
                    nc.scalar.activation(
                        sqsc[:], xt0[:], AF.Square, accum_out=ssx[:, m : m + 1]
                    )
                nrmx = small.tile([P, MT], f32, tag="nrmx")
                nc.scalar.sqrt(nrmx[:], ssx[:])
                nc.vector.reciprocal(invx[:], nrmx[:])

                # ---- prologue B: memory inv-norms + normalize + transpose ----
                ssm = const_pool.tile([P, S // P], f32)
                for s in range(S // P):
                    mt0 = nwork.tile([P, D], f32, tag="nload")
                    nc.sync.dma_start(mt0[:], mem_in[s * P : (s + 1) * P, :])
                    sqsc = nwork.tile([P, D], f32, tag="sqscr")
                    nc.scalar.activation(
                        sqsc[:], mt0[:], AF.Square, accum_out=ssm[:, s : s + 1]
                    )
                nrmm = small.tile([P, S // P], f32, tag="nrmm")
                nc.scalar.sqrt(nrmm[:], ssm[:])
                nc.vector.reciprocal(invm[:], nrmm[:])

                for s in range(S // P):
                    mt1 = nwork.tile([P, D], f32, tag="nload")
                    nc.sync.dma_start(mt1[:], mem_in[s * P : (s + 1) * P, :])
                    mn16 = nwork.tile([P, D], f16_t, tag="mn16")
                    nc.scalar.activation(
                        mn16[:], mt1[:], AF.Copy, scale=invm[:, s : s + 1]
                    )
                    for d in range(DC):
                        pt16 = psum_t16.tile([P, P], f16_t, tag="pt16")
                        nc.tensor.transpose(
                            pt16[:], mn16[:, d * P : (d + 1) * P], ident16[:]
                        )
                        nc.scalar.copy(memT[:, d, s * P : (s + 1) * P], pt16[:])

            # ---- main loop over row tiles ----
            for m in range(MT):
                xt = xwork.tile([P, D], f32, tag="xt")
                nc.sync.dma_start(xt[:], x_in[m * P : (m + 1) * P, :])
                xT = xwork.tile([P, DC, P], f16, tag="xT")
                for d in range(DC):
                    pt = psum_t.tile([P, P], f32, tag="pt")
                    nc.tensor.transpose(pt[:], xt[:, d * P : (d + 1) * P], ident[:])
                    nc.scalar.copy(xT[:, d, :], pt[:])  # cast fp32->fp16

                simrow = sim_pool.tile([P, S], f32, tag="simrow")
                for n in range(NCH):
                    ps = psum_sim.tile([P, 512], f32, tag="ps")
                    for d in range(DC):
                        nc.tensor.matmul(
                            ps[:],
                            lhsT=xT[:, d, :],
                            rhs=memT[:, d, n * 512 : (n + 1) * 512],
                            start=(d == 0),
                            stop=(d == DC - 1),
                        )
                    nc.scalar.copy(simrow[:, n * 512 : (n + 1) * 512], ps[:])

                vals = small.tile([P, K], f32, tag="vals")
                nc.vector.max(out=vals[:], in_=simrow[:])
                idx = small.tile([P, K], u32, tag="idx")
                nc.vector.max_index(out=idx[:], in_max=vals[:], in_values=simrow[:])

                # softmax over cosine scores: cos = raw * invx; fold invx into
                # the exp's scale, bias = -max(raw)*invx
                negmax = small.tile([P, 1], f32, tag="negmax")
                nc.vector.tensor_scalar(
                    negmax[:],
                    vals[:, 0:1],
                    invx[:, m : m + 1],
                    -1.0,
                    op0=OP.mult,
                    op1=OP.mult,
                )
                e8 = small.tile([P, K], f32, tag="e8")
                zsum = small.tile([P, 1], f32, tag="zsum")
                nc.scalar.activation(
                    e8[:],
                    vals[:],
                    AF.Exp,
                    bias=negmax[:, 0:1],
                    scale=invx[:, m : m + 1],
                    accum_out=zsum[:],
                )
                rz = small.tile([P, 1], f32, tag="rz")
                nc.vector.reciprocal(rz[:], zsum[:])
                w8 = small.tile([P, K], f32, tag="w8")
                nc.vector.tensor_scalar(
                    w8[:], e8[:], rz[:, 0:1], float(scale), op0=OP.mult, op1=OP.mult
                )

                retr = retr_pool.tile([P, K, D], f32, tag="retr")
                if MERGED_GATHER:
                    nc.gpsimd.indirect_dma_start(
                        out=retr[:, :, :],
                        out_offset=None,
                        in_=mem_in[:],
                        in_offset=bass.IndirectOffsetOnAxis(ap=idx[:, :], axis=0),
                    )
                else:
                    for k in range(K):
                        nc.gpsimd.indirect_dma_start(
                            out=retr[:, k, :],
                            out_offset=None,
                            in_=mem_in[:],
                            in_offset=bass.IndirectOffsetOnAxis(
                                ap=idx[:, k : k + 1], axis=0
                            ),
                        )
                for k in range(K):
                    nc.scalar.activation(
                        retr[:, k, :], retr[:, k, :], AF.Copy, scale=w8[:, k : k + 1]
                    )
                nc.gpsimd.tensor_add(retr[:, 0:4, :], retr[:, 0:4, :], retr[:, 4:8, :])
                nc.gpsimd.tensor_add(retr[:, 0:2, :], retr[:, 0:2, :], retr[:, 2:4, :])
                nc.gpsimd.tensor_add(retr[:, 0:1, :], retr[:, 0:1, :], retr[:, 1:2, :])
                nc.gpsimd.tensor_add(xt[:], xt[:], retr[:, 0, :])
                nc.sync.dma_start(out_t[m * P : (m + 1) * P, :], xt[:])

    return nc


def _get_program(scale: float, n_row_tiles: int = ROWS // P):
    key = (float(scale), n_row_tiles)
    if key not in _cache:
        nc = build_program(float(scale), n_row_tiles)
        if not nc.is_finalized():
            nc.finalize()  # runs the Bacc lowering pipeline (wait splitting etc.)
        _cache[key] = nc
    return _cache[key]


def kernel(x: np.ndarray, memory: np.ndarray, retrieval_scale) -> np.ndarray:
    from concourse.bass_utils import run_bass_kernel_spmd

    x = np.ascontiguousarray(x, dtype=np.float32)
    memory = np.ascontiguousarray(memory, dtype=np.float32)
    scale = float(np.asarray(retrieval_scale))

    nc = _get_program(scale)
    in_maps = [
        {"x": x[b].reshape(ROWS, D), "memory": memory} for b in range(x.shape[0])
    ]
    res = run_bass_kernel_spmd(nc, in_maps, core_ids=list(range(x.shape[0])))
    global last_results
    last_results = res
    out = np.stack([r["out"] for r in res.results]).reshape(x.shape)
    return out.astype(np.float32)


# revision 30
# speedup vs baseline: 1.5996x; 1.4304x over previous
"""BPBookMemory retrieval-knn kernel for 8 Trainium2 NeuronCores.

Per core (data-parallel over batch: core b handles x[b] [4096, 512]):
  1. Prologue: row inv-norms of x and memory (ACT square+accum, one sqrt,
     one reciprocal); normalized memory transposed into fp16
     memT [128, 4, 8192] (SBUF-resident; DVE drains the transposes).
  2. Stage 1 per 128-row x tile: PE-transpose raw x (ranking is invariant
     to row scale; 1/||x|| is folded into the softmax), raw scores =
     x @ mem_n^T via fp16 matmuls (fp16 mantissa == tf32; FWL hides
     LDWEIGHTS; d-outer n-groups share weights across 6 PSUM banks),
     ACT drains PSUM->SBUF, DVE max/max_index -> top-8 scores + indices.
  3. Stage 2 (emitted 2 tiles behind stage 1 so the in-order engine
     queues never head-of-line block): gpsimd indirect gather (cast to
     fp16), ACT softmax + per-k scaling, gpsimd tree-add + residual add,
     DMA out.
"""

import sys

import numpy as np

for _p in ("/opt/trn_rl_repo",):
    if _p not in sys.path:
        sys.path.insert(0, _p)

P = 128
D = 512
S = 8192
K = 8
ROWS = 4096  # rows per core (one batch element)
DC = D // P  # 4 contraction chunks
NCH = S // 512  # 16 sim column chunks
B = 8
LAG = 2  # software-pipeline distance between stage1 and stage2
N_GROUPS = (6, 6, 4)  # PSUM banks per matmul group (sum = NCH)
SIM_BUFS = 2
RETR_BUFS = 2

_cache = {}
last_results = None  # BassKernelResults of most recent kernel() call


def build_program(scale: float, n_row_tiles: int = ROWS // P):
    import concourse.bass as bass
    import concourse.mybir as mybir
    import concourse.tile as tile
    from concourse import bacc
    from concourse.masks import make_identity

    f32 = mybir.dt.float32
    f16 = mybir.dt.float16
    u32 = mybir.dt.uint32
    AF = mybir.ActivationFunctionType
    OP = mybir.AluOpType
    MT = n_row_tiles

    nc = bacc.Bacc()
    x_in = nc.dram_tensor("x", [MT * P, D], f32, kind="ExternalInput")
    mem_in = nc.dram_tensor("memory", [S, D], f32, kind="ExternalInput")
    out_t = nc.dram_tensor("out", [MT * P, D], f32, kind="ExternalOutput")

    with tile.TileContext(nc) as tc:
        with (
            tc.tile_pool(name="const", bufs=1) as const_pool,
            tc.tile_pool(name="xld", bufs=LAG + 2) as xld,
            tc.tile_pool(name="xwork", bufs=2) as xwork,
            tc.tile_pool(name="sim", bufs=SIM_BUFS) as sim_pool,
            tc.tile_pool(name="retr", bufs=RETR_BUFS) as retr_pool,
            tc.tile_pool(name="small", bufs=LAG + 2) as small,
            tc.tile_pool(name="psum_t", bufs=2, space="PSUM") as psum_t,
        ):
            ident = const_pool.tile([P, P], f32)
            make_identity(nc, ident[:])
            ident16 = const_pool.tile([P, P], f16)
            nc.vector.tensor_copy(ident16[:], ident[:])
            # PE instructions carry at most ONE wait cheaply; absorb the Pool
            # (make_identity) dep into PE's observed clock via a dummy transpose.
            scratch = const_pool.tile([P, P], f32)
            pt0 = psum_t.tile([P, P], f32, tag="pt")
            nc.tensor.transpose(pt0[:], ident[:], ident[:])
            nc.scalar.copy(scratch[:], pt0[:])

            memT = const_pool.tile([P, DC, S], f16)
            invx = const_pool.tile([P, MT], f32)
            invm = const_pool.tile([P, S // P], f32)
            ssx = const_pool.tile([P, MT], f32)
            ssm = const_pool.tile([P, S // P], f32)

            with (
                tc.tile_pool(name="nwork", bufs=3) as nwork,
                tc.tile_pool(name="psum_t16", bufs=2, space="PSUM") as psum_t16,
            ):
                scratch16 = const_pool.tile([P, P], f16)
                pt016 = psum_t16.tile([P, P], f16, tag="pt16")
                nc.tensor.transpose(pt016[:], ident16[:], ident16[:])
                nc.vector.tensor_copy(scratch16[:], pt016[:])

                # ---- prologue A: x row inv-norms (cheap extra pass) ----
                for m in range(MT):
                    xt0 = nwork.tile([P, D], f32, tag="nload")
                    nc.sync.dma_start(xt0[:], x_in[m * P : (m + 1) * P, :])
                    sqsc = nwork.tile([P, D], f32, tag="sqscr")
                    nc.scalar.activation(
                        sqsc[:], xt0[:], AF.Square, accum_out=ssx[:, m : m + 1]
                    )
                nrmx = const_pool.tile([P, MT], f32)
                nc.scalar.sqrt(nrmx[:], ssx[:])
                nc.vector.reciprocal(invx[:], nrmx[:])

                # ---- prologue B: memory inv-norms ----
                for s in range(S // P):
                    mt0 = nwork.tile([P, D], f32, tag="nload")
                    nc.sync.dma_start(mt0[:], mem_in[s * P : (s + 1) * P, :])
                    sqsc = nwork.tile([P, D], f32, tag="sqscr")
                    nc.scalar.activation(
                        sqsc[:], mt0[:], AF.Square, accum_out=ssm[:, s : s + 1]
                    )
                nrmm = const_pool.tile([P, S // P], f32)
                nc.scalar.sqrt(nrmm[:], ssm[:])
                nc.vector.reciprocal(invm[:], nrmm[:])

                # ---- prologue C: normalize + transpose memory (fp16) ----
                for s in range(S // P):
                    mt1 = nwork.tile([P, D], f32, tag="nload")
                    nc.sync.dma_start(mt1[:], mem_in[s * P : (s + 1) * P, :])
                    mn16 = nwork.tile([P, D], f16, tag="mn16")
                    nc.scalar.activation(
                        mn16[:], mt1[:], AF.Copy, scale=invm[:, s : s + 1]
                    )
                    for d in range(DC):
                        pt16 = psum_t16.tile([P, P], f16, tag="pt16")
                        nc.tensor.transpose(
                            pt16[:], mn16[:, d * P : (d + 1) * P], ident16[:]
                        )
                        nc.vector.tensor_copy(memT[:, d, s * P : (s + 1) * P], pt16[:])

            with tc.tile_pool(
                name="psum_sim", bufs=max(N_GROUPS), space="PSUM"
            ) as psum_sim:
                xts = {}
                stash = {}

                def stage1(m):
                    xt = xld.tile([P, D], f32, tag="xt")
                    xts[m] = xt
                    nc.sync.dma_start(xt[:], x_in[m * P : (m + 1) * P, :])
                    xT = xwork.tile([P, DC, P], f16, tag="xT")
                    for d in range(DC):
                        pt = psum_t.tile([P, P], f32, tag="pt")
                        nc.tensor.transpose(pt[:], xt[:, d * P : (d + 1) * P], ident[:])
                        nc.scalar.copy(xT[:, d, :], pt[:])  # cast fp32->fp16

                    simrow = sim_pool.tile([P, S], f32, tag="simrow")
                    n0 = 0
                    for gsize in N_GROUPS:
                        banks = [
                            psum_sim.tile([P, 512], f32, tag="ps", name="ps")
                            for _ in range(gsize)
                        ]
                        for d in range(DC):
                            for j in range(gsize):
                                nc.tensor.matmul(
                                    banks[j][:],
                                    lhsT=xT[:, d, :],
                                    rhs=memT[:, d, (n0 + j) * 512 : (n0 + j + 1) * 512],
                                    start=(d == 0),
                                    stop=(d == DC - 1),
                                )
                        for j in range(gsize):
                            nc.scalar.copy(
                                simrow[:, (n0 + j) * 512 : (n0 + j + 1) * 512],
                                banks[j][:],
                            )
                        n0 += gsize

                    vals = small.tile([P, K], f32, tag="vals")
                    nc.vector.max(out=vals[:], in_=simrow[:])
                    idx = small.tile([P, K], u32, tag="idx")
                    nc.vector.max_index(out=idx[:], in_max=vals[:], in_values=simrow[:])
                    stash[m] = (vals, idx)

                def stage2(m):
                    vals, idx = stash.pop(m)
                    xt = xts.pop(m)
                    retr = retr_pool.tile([P, K, D], f16, tag="retr")
                    for k in range(K):
                        nc.gpsimd.indirect_dma_start(
                            out=retr[:, k, :],
                            out_offset=None,
                            in_=mem_in[:],
                            in_offset=bass.IndirectOffsetOnAxis(
                                ap=idx[:, k : k + 1], axis=0
                            ),
                        )
                    negmax = small.tile([P, 1], f32, tag="negmax")
                    nc.vector.tensor_scalar(
                        negmax[:],
                        vals[:, 0:1],
                        invx[:, m : m + 1],
                        -1.0,
                        op0=OP.mult,
                        op1=OP.mult,
                    )
                    e8 = small.tile([P, K], f32, tag="e8")
                    zsum = small.tile([P, 1], f32, tag="zsum")
                    nc.scalar.activation(
                        e8[:],
                        vals[:],
                        AF.Exp,
                        bias=negmax[:, 0:1],
                        scale=invx[:, m : m + 1],
                        accum_out=zsum[:],
                    )
                    rz = small.tile([P, 1], f32, tag="rz")
                    nc.vector.reciprocal(rz[:], zsum[:])
                    w8 = small.tile([P, K], f32, tag="w8")
                    nc.vector.tensor_scalar(
                        w8[:], e8[:], rz[:, 0:1], float(scale), op0=OP.mult, op1=OP.mult
                    )
                    for k in range(K):
                        nc.scalar.activation(
                            retr[:, k, :],
                            retr[:, k, :],
                            AF.Copy,
                            scale=w8[:, k : k + 1],
                        )
                    nc.gpsimd.tensor_add(retr[:, 0:4, :], retr[:, 0:4, :], retr[:, 4:8, :])
                    nc.gpsimd.tensor_add(retr[:, 0:2, :], retr[:, 0:2, :], retr[:, 2:4, :])
                    nc.gpsimd.tensor_add(retr[:, 0:1, :], retr[:, 0:1, :], retr[:, 1:2, :])
                    nc.gpsimd.tensor_add(xt[:], xt[:], retr[:, 0, :])
                    nc.sync.dma_start(out_t[m * P : (m + 1) * P, :], xt[:])

                for m in range(MT + LAG):
                    if m < MT:
                        stage1(m)
                    if m >= LAG:
                        stage2(m - LAG)

    return nc


def _get_program(scale: float, n_row_tiles: int = ROWS // P):
    key = (float(scale), n_row_tiles)
    if key not in _cache:
        nc = build_program(float(scale), n_row_tiles)
        if not nc.is_finalized():
            nc.finalize()  # runs the Bacc lowering pipeline (wait splitting etc.)
        _cache[key] = nc
    return _cache[key]


def kernel(x: np.ndarray, memory: np.ndarray, retrieval_scale) -> np.ndarray:
    from concourse.bass_utils import run_bass_kernel_spmd

    x = np.ascontiguousarray(x, dtype=np.float32)
    memory = np.ascontiguousarray(memory, dtype=np.float32)
    scale = float(np.asarray(retrieval_scale))

    nc = _get_program(scale)
    in_maps = [
        {"x": x[b].reshape(ROWS, D), "memory": memory} for b in range(x.shape[0])
    ]
    res = run_bass_kernel_spmd(nc, in_maps, core_ids=list(range(x.shape[0])))
    global last_results
    last_results = res
    out = np.stack([r["out"] for r in res.results]).reshape(x.shape)
    return out.astype(np.float32)
